# revision 52
# baseline (speedup 1.0000x reference)
"""Trainium2 Bass kernel for ExpertMLPLoRA (moe_routing).

Reference computation (per batch b, selected expert k):
    A = A_all[expert_indices]            # [K, D, R]
    Bm = B_all[expert_indices]           # [K, R, D]
    down = einsum('bkmd,kdr->bkmr', z, A)
    up   = einsum('bkmr,krd->bkmd', down, Bm)
    out  = up * (alpha/rank)

Sharding: data-parallel over batch B=8 -> one batch per NeuronCore.
Each core receives its z[b] slice plus the full (replicated) LoRA
tables and gathers the K=8 selected experts on-device via indirect
DMA.  Host only pre-expands the 8 expert indices into row indices
(pure address arithmetic).

Device pipeline per (b, k):
  1. SWDGE cast-DMA z[b,k] [512, 1024] f32 HBM -> bf16 SBUF [128p, (mc,d)]
  2. 32x PE transpose (bf16, via identity matmul) -> z^T chunks in PSUM,
     copied back to SBUF alternating DVE/ACT
  3. mm1: one 8-matmul PSUM accumulation group (start/stop flags) ->
     down^T [16r, 512m] f32 in one PSUM bank; DVE casts to bf16.
     (The accumulation group is contiguous in the PE stream - no
     foreign matmuls interleave, so the hw accumulation fault the old
     partial-add tree worked around cannot trigger.)
  4. mm2: 8 matmuls [16,128m].T @ B_k[16,512d] -> f32 PSUM
  5. PSUM -> SBUF f32 copies (alternating DVE/ACT), SWDGE store per
     128-row strip.

The LoRA scale folds into the bf16 cast of the gathered A table.
"""

import numpy as np

_B, _K, _M, _D, _R = 8, 8, 512, 1024, 16
_SCALE = 1.0 / _R
_NCORES = 8

_cache = {}


def _apply_tile_drain_patch():
    """This walrus build caps sync waits at 1 per instruction (2 for
    EventSemaphore).  Tile's kernel-tail drain piles every final sem wait
    onto one Drain -> NCC_INLA001 'Too many sync wait commands'.  Re-emit
    the extras as standalone per-sem waits before the drain."""
    import concourse.tile as tile_mod
    from concourse.tile import TileContext

    if getattr(TileContext, "_drain_patch_applied", False):
        return
    try:
        from concourse.tile import ScopedClock
    except ImportError:
        from bass_rust import ScopedClock

    def _patched(self, tick_clock, wait_clock):
        nc = self.nc
        probe = nc.sync.drain()
        wait_clock.add_sem_waits(
            probe.ins, ScopedClock({None: tick_clock.global_clock})
        )
        waits = list(probe.ins.sync_info.on_wait)
        if len(waits) > 1:
            assert self.sems is not None
            by_name = {s.name: s for s in self.sems.allocated().values()}
            for w in waits[1:]:
                sem = by_name.get(w.ant_name)
                assert sem is not None, f"semaphore {w.ant_name} not found"
                nc.sync.wait_ge(sem, w.wait_value)
            probe.ins.sync_info.on_wait = waits[:1]
            nc.sync.drain()
        nc.all_engine_barrier()
        assert self.sems is not None
        popped = nc._tile_sem_poison_stack.pop()
        assert popped is self._sem_poison
        nc.clear_and_free_semaphores(list(self.sems.allocated().values()))
        nc.all_engine_barrier()

    TileContext._drain_and_barrier = _patched
    TileContext._drain_patch_applied = True


def _split_excess_waits(nc):
    """This walrus build rejects instructions carrying more than 1-2 sync
    waits ('Too many sync wait commands'), but Tile's sem-assignment packs
    up to ~9 waits onto one instruction.  Hoist the excess onto standalone
    EventSemaphore carriers placed immediately before the instruction on
    the same engine (engines execute in order, so blocking semantics are
    identical)."""
    import bass_rust
    import concourse.mybir as mybir

    n = 0
    for fn in nc.m.functions:
        for bb in fn.blocks:
            new_insts = []
            for inst in bb.instructions:
                si = inst.sync_info
                waits = list(si.on_wait) if si is not None else []
                cap = 2 if isinstance(inst, mybir.InstEventSemaphore) else 1
                if len(waits) > cap:
                    for w in waits[cap:]:
                        n += 1
                        new_insts.append(
                            mybir.InstEventSemaphore(
                                name=f"wsplit-{n}-{inst.name}",
                                engine=inst.engine,
                                ins=[],
                                outs=[],
                                sync_info=bass_rust.SyncInfo(
                                    on_wait=[w], on_update=[]
                                ),
                            )
                        )
                    inst.sync_info = bass_rust.SyncInfo(
                        on_wait=waits[:cap], on_update=list(si.on_update)
                    )
                new_insts.append(inst)
            bb.instructions = new_insts
    return n


def _build(split_waits=True):
    import concourse.bass as bass
    import concourse.mybir as mybir
    from concourse.masks import make_identity
    from concourse.tile import TileContext

    _apply_tile_drain_patch()
    f32 = mybir.dt.float32
    bf16 = mybir.dt.bfloat16
    i32 = mybir.dt.int32

    nc = bass.Bass()
    z = nc.declare_dram_parameter("z", [_K, _M, _D], f32, isOutput=False)
    # A_all [64, 1024, 16] viewed as rows (e, dc) of [128, 16] blocks
    a_tab = nc.declare_dram_parameter("a_tab", [64 * 8, 128 * _R], f32, isOutput=False)
    # B_all [64, 16, 1024] viewed as rows (e, r) of [1024] d-vectors
    b_tab = nc.declare_dram_parameter("b_tab", [64 * _R, _D], f32, isOutput=False)
    idxa = nc.declare_dram_parameter("idxa", [64, 1], i32, isOutput=False)
    idxb = nc.declare_dram_parameter("idxb", [128, 1], i32, isOutput=False)
    out = nc.declare_dram_parameter("out", [_K, _M, _D], f32, isOutput=True)

    with TileContext(nc) as tc:
        with (
            tc.tile_pool(name="const", bufs=1) as cpool,
            tc.tile_pool(name="zbp", bufs=8) as zbpool,
            tc.tile_pool(name="ztp", bufs=2) as ztpool,
            tc.tile_pool(name="ovp", bufs=5) as ovpool,
            tc.tile_pool(name="acc", bufs=2) as apool,
            tc.tile_pool(name="psd", bufs=2, space="PSUM") as psd,
            tc.tile_pool(name="psu", bufs=2, space="PSUM") as psu,
            tc.tile_pool(name="pst", bufs=2, space="PSUM") as pst,
        ):
            def load_zb(k):
                # SWDGE cast-DMA: f32 HBM -> bf16 SBUF in one transfer
                zb = zbpool.tile([128, 4096], bf16, tag="zb")
                nc.gpsimd.dma_start(
                    out=zb[:].rearrange("p (mc d) -> p mc d", mc=4),
                    in_=z[k].rearrange("(mc p) d -> p mc d", p=128),
                )
                return zb

            ident = cpool.tile([128, 128], bf16)
            make_identity(nc, ident[:])

            # first z load starts before any gather work occupies Q7
            zb_pre = [load_zb(0)]

            # ---- one-time expert gather + layout prep ----
            ia = cpool.tile([64, 1], i32)
            nc.sync.dma_start(out=ia[:], in_=idxa[:])

            # gather A rows (k,dc) -> [64, 2048]; row content is [128p, 16r]
            a_rows = cpool.tile([64, 2048], f32)
            nc.gpsimd.indirect_dma_start(
                out=a_rows[:],
                out_offset=None,
                in_=a_tab[:],
                in_offset=bass.IndirectOffsetOnAxis(ap=ia[:, :1], axis=0),
            )
            # identity for PE transposes (f32, A-table prep only)
            identf = cpool.tile([128, 128], f32)
            make_identity(nc, identf[:])
            # redistribute d across partitions with 16 strided PE
            # transposes (one per rank index r): [64(k,dc), 128(d)] -> psum
            # [128(d), (r, k, dc)], then one free-dim-permuted DVE copy with
            # the LoRA scale and bf16 cast folded in.
            a_rows_v = a_rows[:].rearrange("j (p r) -> j r p", r=_R)
            a_tb = cpool.tile([128, 8 * 8 * _R], bf16)
            a_tb_v = a_tb[:].rearrange("p (k dc r) -> p r (k dc)", k=8, r=_R)
            for h in range(2):
                pa = psu.tile([128, 512], f32, tag="up")
                for rr in range(8):
                    nc.tensor.transpose(
                        out=pa[:, rr * 64 : (rr + 1) * 64],
                        in_=a_rows_v[:, h * 8 + rr, :],
                        identity=identf[:64, :64],
                    )
                nc.vector.tensor_scalar_mul(
                    a_tb_v[:, h * 8 : (h + 1) * 8, :],
                    pa[:].rearrange("p (r j) -> p r j", r=8),
                    _SCALE,
                )

            # gather ALL 128 B rows (k,r) in ONE casting indirect DMA
            # (keeps the Q7 descriptor-generation path clear for z loads),
            # then shift each expert's 16 rows to partition base 0 with
            # tiny HWDGE SBUF->SBUF copies (matmul operands must sit at
            # SBUF base partition 0; engines can't shift partitions).
            ib = cpool.tile([128, 1], i32)
            nc.sync.dma_start(out=ib[:], in_=idxb[:])
            ball = cpool.tile([128, _D], bf16)
            nc.gpsimd.indirect_dma_start(
                out=ball[:],
                out_offset=None,
                in_=b_tab[:],
                in_offset=bass.IndirectOffsetOnAxis(ap=ib[:, :1], axis=0),
            )
            b_kt = []
            for k in range(_K):
                btb = cpool.tile([16, _D], bf16, tag=f"bb{k}")
                eng = nc.sync if k % 2 == 0 else nc.scalar
                eng.dma_start(out=btb[:], in_=ball[16 * k : 16 * (k + 1), :])
                b_kt.append(btb)

            # enqueue every remaining z load ahead of all stores in the
            # SWDGE descriptor stream: loads then run back-to-back at
            # full HBM bandwidth while the store backlog drains behind
            zb_pre += [load_zb(k) for k in range(1, _K)]

            # z^T chunks via PE transpose (bf16, 1 cycle/row):
            #   zt[p, dc*512+m] = z[k, m, dc*128+p]
            # Emitted in two halves so the per-k transpose block can be
            # software-pipelined between the previous k's mm1 and mm2:
            # the PE never sees a >2us run of transpose-mode work, which
            # keeps the HAM clock gate warm (transpose-mode doesn't count
            # as PE-busy for HAM).
            zts = {}

            def transposes(k, half):
                zb, zt = zb_pre[k], zts[k]
                for dh in (0, 1) if half == 0 else (2, 3):
                    pt = pst.tile([128, 1024], bf16, tag="zt_ps")
                    for dj in range(2):
                        dc = dh * 2 + dj
                        for mc in range(4):
                            nc.tensor.transpose(
                                out=pt[:, dj * 512 + mc * 128 : dj * 512 + (mc + 1) * 128],
                                in_=zb[:, mc * 1024 + dc * 128 : mc * 1024 + (dc + 1) * 128],
                                identity=ident[:],
                            )
                    dst = zt[:, dh * 1024 : (dh + 1) * 1024]
                    if dh % 2 == 0:
                        nc.vector.tensor_copy(out=dst, in_=pt[:])
                    else:
                        nc.scalar.copy(out=dst, in_=pt[:])

            # ---- main loop over the K selected experts ----
            zt0 = ztpool.tile([128, 4096], bf16, tag="zt")
            zts[0] = zt0
            transposes(0, 0)
            transposes(0, 1)
            for k in range(_K):
                zt = zts[k]
                if k + 1 < _K:
                    ztn = ztpool.tile([128, 4096], bf16, tag="zt")
                    zts[k + 1] = ztn
                    transposes(k + 1, 0)

                # mm1: down^T [16, 512] via one 8-matmul PSUM accumulation
                # group (contiguous in the PE stream - no foreign matmuls
                # can interleave, so hw accumulation is safe here)
                pd = psd.tile([16, _M], f32, tag="down")
                for dc in range(8):
                    nc.tensor.matmul(
                        out=pd[:],
                        lhsT=a_tb[:, (k * 8 + dc) * _R : (k * 8 + dc + 1) * _R],
                        rhs=zt[:, dc * 512 : (dc + 1) * 512],
                        start=(dc == 0),
                        stop=(dc == 7),
                    )
                if k + 1 < _K:
                    transposes(k + 1, 1)

                db = apool.tile([16, _M], bf16, tag="db")
                nc.vector.tensor_copy(out=db[:], in_=pd[:])

                # mm2 + copy out.  f32 staging; stores go out on the HWDGE
                # queues, whose completion semaphores are incremented by
                # the DMA hardware itself - SWDGE completion is noticed by
                # Q7 ucode on a ~10us polling cadence once idle, which put
                # a stochastic 10-30us wait on the final drain.
                ov = ovpool.tile([128, 4096], f32, tag="ov")
                for mc in range(4):
                    for dh in range(2):
                        pu = psu.tile([128, 512], f32, tag="up")
                        nc.tensor.matmul(
                            out=pu[:],
                            lhsT=db[:, mc * 128 : (mc + 1) * 128],
                            rhs=b_kt[k][:, dh * 512 : (dh + 1) * 512],
                            start=True,
                            stop=True,
                        )
                        dst = ov[:, mc * 1024 + dh * 512 : mc * 1024 + (dh + 1) * 512]
                        if (mc * 2 + dh) % 2 == 0:
                            nc.vector.tensor_copy(out=dst, in_=pu[:])
                        else:
                            nc.scalar.copy(out=dst, in_=pu[:])
                seng = nc.sync if k % 2 == 0 else nc.scalar
                seng.dma_start(
                    out=out[k].rearrange("(mc p) d -> p mc d", p=128),
                    in_=ov[:].rearrange("p (mc d) -> p mc d", mc=4),
                )
    if split_waits:
        _split_excess_waits(nc)
    return nc


def kernel(z, A_all, B_all, expert_indices, _trace=False):
    from concourse.bass_utils import run_bass_kernel_spmd

    z = np.ascontiguousarray(np.asarray(z, dtype=np.float32))
    A_all = np.ascontiguousarray(np.asarray(A_all, dtype=np.float32))
    B_all = np.ascontiguousarray(np.asarray(B_all, dtype=np.float32))
    idx = np.asarray(expert_indices).astype(np.int64)
    assert z.shape == (_B, _K, _M, _D)

    if "nc" not in _cache:
        _cache["nc"] = _build()
    nc = _cache["nc"]

    a_tab = A_all.reshape(64 * 8, 128 * _R)
    b_tab = B_all.reshape(64 * _R, _D)
    idxa = (idx[:, None] * 8 + np.arange(8)[None, :]).reshape(64, 1).astype(np.int32)
    idxb = (idx[:, None] * 16 + np.arange(16)[None, :]).reshape(128, 1).astype(np.int32)

    in_maps = [
        {"z": z[c], "a_tab": a_tab, "b_tab": b_tab, "idxa": idxa, "idxb": idxb}
        for c in range(_NCORES)
    ]
    res = run_bass_kernel_spmd(nc, in_maps, list(range(_NCORES)), trace=_trace)
    globals()["last_exec_time_ns"] = res.exec_time_ns
    return np.stack([res.results[c]["out"] for c in range(_NCORES)], axis=0)


# revision 54
# speedup vs baseline: 1.0265x; 1.0265x over previous
"""Trainium2 Bass kernel for ExpertMLPLoRA (moe_routing).

Reference computation (per batch b, selected expert k):
    A = A_all[expert_indices]            # [K, D, R]
    Bm = B_all[expert_indices]           # [K, R, D]
    down = einsum('bkmd,kdr->bkmr', z, A)
    up   = einsum('bkmr,krd->bkmd', down, Bm)
    out  = up * (alpha/rank)

Sharding: data-parallel over batch B=8 -> one batch per NeuronCore.
Each core receives its z[b] slice plus the full (replicated) LoRA
tables and gathers the K=8 selected experts on-device via indirect
DMA.  Host only pre-expands the 8 expert indices into row indices
(pure address arithmetic).

Device pipeline per (b, k):
  1. SWDGE cast-DMA z[b,k] [512, 1024] f32 HBM -> bf16 SBUF [128p, (mc,d)]
  2. 32x PE transpose (bf16, via identity matmul) -> z^T chunks in PSUM,
     copied back to SBUF alternating DVE/ACT
  3. mm1: one 8-matmul PSUM accumulation group (start/stop flags) ->
     down^T [16r, 512m] f32 in one PSUM bank; DVE casts to bf16.
     (The accumulation group is contiguous in the PE stream - no
     foreign matmuls interleave, so the hw accumulation fault the old
     partial-add tree worked around cannot trigger.)
  4. mm2: 8 matmuls [16,128m].T @ B_k[16,512d] -> f32 PSUM
  5. PSUM -> SBUF f32 copies (alternating DVE/ACT), SWDGE store per
     128-row strip.

The LoRA scale folds into the bf16 cast of the gathered A table.
"""

import numpy as np

_B, _K, _M, _D, _R = 8, 8, 512, 1024, 16
_SCALE = 1.0 / _R
_NCORES = 8

_cache = {}


def _apply_tile_drain_patch():
    """This walrus build caps sync waits at 1 per instruction (2 for
    EventSemaphore).  Tile's kernel-tail drain piles every final sem wait
    onto one Drain -> NCC_INLA001 'Too many sync wait commands'.  Re-emit
    the extras as standalone per-sem waits before the drain."""
    import concourse.tile as tile_mod
    from concourse.tile import TileContext

    if getattr(TileContext, "_drain_patch_applied", False):
        return
    try:
        from concourse.tile import ScopedClock
    except ImportError:
        from bass_rust import ScopedClock

    def _patched(self, tick_clock, wait_clock):
        nc = self.nc
        probe = nc.sync.drain()
        wait_clock.add_sem_waits(
            probe.ins, ScopedClock({None: tick_clock.global_clock})
        )
        waits = list(probe.ins.sync_info.on_wait)
        if len(waits) > 1:
            assert self.sems is not None
            by_name = {s.name: s for s in self.sems.allocated().values()}
            for w in waits[1:]:
                sem = by_name.get(w.ant_name)
                assert sem is not None, f"semaphore {w.ant_name} not found"
                nc.sync.wait_ge(sem, w.wait_value)
            probe.ins.sync_info.on_wait = waits[:1]
            nc.sync.drain()
        nc.all_engine_barrier()
        assert self.sems is not None
        popped = nc._tile_sem_poison_stack.pop()
        assert popped is self._sem_poison
        nc.clear_and_free_semaphores(list(self.sems.allocated().values()))
        nc.all_engine_barrier()

    TileContext._drain_and_barrier = _patched
    TileContext._drain_patch_applied = True


def _split_excess_waits(nc):
    """This walrus build rejects instructions carrying more than 1-2 sync
    waits ('Too many sync wait commands'), but Tile's sem-assignment packs
    up to ~9 waits onto one instruction.  Hoist the excess onto standalone
    EventSemaphore carriers placed immediately before the instruction on
    the same engine (engines execute in order, so blocking semantics are
    identical)."""
    import bass_rust
    import concourse.mybir as mybir

    n = 0
    for fn in nc.m.functions:
        for bb in fn.blocks:
            new_insts = []
            for inst in bb.instructions:
                si = inst.sync_info
                waits = list(si.on_wait) if si is not None else []
                cap = 2 if isinstance(inst, mybir.InstEventSemaphore) else 1
                if len(waits) > cap:
                    for w in waits[cap:]:
                        n += 1
                        new_insts.append(
                            mybir.InstEventSemaphore(
                                name=f"wsplit-{n}-{inst.name}",
                                engine=inst.engine,
                                ins=[],
                                outs=[],
                                sync_info=bass_rust.SyncInfo(
                                    on_wait=[w], on_update=[]
                                ),
                            )
                        )
                    inst.sync_info = bass_rust.SyncInfo(
                        on_wait=waits[:cap], on_update=list(si.on_update)
                    )
                new_insts.append(inst)
            bb.instructions = new_insts
    return n


def _build(split_waits=True):
    import concourse.bass as bass
    import concourse.mybir as mybir
    from concourse.masks import make_identity
    from concourse.tile import TileContext

    _apply_tile_drain_patch()
    f32 = mybir.dt.float32
    bf16 = mybir.dt.bfloat16
    i32 = mybir.dt.int32

    nc = bass.Bass()
    z = nc.declare_dram_parameter("z", [_K, _M, _D], f32, isOutput=False)
    # A_all [64, 1024, 16] viewed as rows (e, dc) of [128, 16] blocks
    a_tab = nc.declare_dram_parameter("a_tab", [64 * 8, 128 * _R], f32, isOutput=False)
    # B_all [64, 16, 1024] viewed as rows (e, r) of [1024] d-vectors
    b_tab = nc.declare_dram_parameter("b_tab", [64 * _R, _D], f32, isOutput=False)
    idxa = nc.declare_dram_parameter("idxa", [64, 1], i32, isOutput=False)
    idxb = nc.declare_dram_parameter("idxb", [128, 1], i32, isOutput=False)
    out = nc.declare_dram_parameter("out", [_K, _M, _D], f32, isOutput=True)

    with TileContext(nc) as tc:
        with (
            tc.tile_pool(name="const", bufs=1) as cpool,
            tc.tile_pool(name="zbp", bufs=8) as zbpool,
            tc.tile_pool(name="ztp", bufs=2) as ztpool,
            tc.tile_pool(name="ovp", bufs=5) as ovpool,
            tc.tile_pool(name="acc", bufs=2) as apool,
            tc.tile_pool(name="psd", bufs=2, space="PSUM") as psd,
            tc.tile_pool(name="psu", bufs=2, space="PSUM") as psu,
            tc.tile_pool(name="pst", bufs=2, space="PSUM") as pst,
        ):
            def load_zb(k):
                # SWDGE cast-DMA: f32 HBM -> bf16 SBUF in one transfer
                zb = zbpool.tile([128, 4096], bf16, tag="zb")
                nc.gpsimd.dma_start(
                    out=zb[:].rearrange("p (mc d) -> p mc d", mc=4),
                    in_=z[k].rearrange("(mc p) d -> p mc d", p=128),
                )
                return zb

            ident = cpool.tile([128, 128], bf16)
            make_identity(nc, ident[:])

            # first z load starts before any gather work occupies Q7
            zb_pre = [load_zb(0)]

            # ---- one-time expert gather + layout prep ----
            ia = cpool.tile([64, 1], i32)
            nc.sync.dma_start(out=ia[:], in_=idxa[:])

            # gather A rows (k,dc) -> [64, 2048]; row content is [128p, 16r]
            a_rows = cpool.tile([64, 2048], f32)
            nc.gpsimd.indirect_dma_start(
                out=a_rows[:],
                out_offset=None,
                in_=a_tab[:],
                in_offset=bass.IndirectOffsetOnAxis(ap=ia[:, :1], axis=0),
            )
            # identity for PE transposes (f32, A-table prep only)
            identf = cpool.tile([128, 128], f32)
            make_identity(nc, identf[:])
            # redistribute d across partitions with 16 strided PE
            # transposes (one per rank index r): [64(k,dc), 128(d)] -> psum
            # [128(d), (r, k, dc)], then one free-dim-permuted DVE copy with
            # the LoRA scale and bf16 cast folded in.
            a_rows_v = a_rows[:].rearrange("j (p r) -> j r p", r=_R)
            a_tb = cpool.tile([128, 8 * 8 * _R], bf16)
            a_tb_v = a_tb[:].rearrange("p (k dc r) -> p r (k dc)", k=8, r=_R)
            for h in range(2):
                pa = psu.tile([128, 512], f32, tag="up")
                for rr in range(8):
                    nc.tensor.transpose(
                        out=pa[:, rr * 64 : (rr + 1) * 64],
                        in_=a_rows_v[:, h * 8 + rr, :],
                        identity=identf[:64, :64],
                    )
                nc.vector.tensor_scalar_mul(
                    a_tb_v[:, h * 8 : (h + 1) * 8, :],
                    pa[:].rearrange("p (r j) -> p r j", r=8),
                    _SCALE,
                )

            # gather ALL 128 B rows (k,r) in ONE casting indirect DMA
            # (keeps the Q7 descriptor-generation path clear for z loads),
            # then shift each expert's 16 rows to partition base 0 with
            # tiny HWDGE SBUF->SBUF copies (matmul operands must sit at
            # SBUF base partition 0; engines can't shift partitions).
            ib = cpool.tile([128, 1], i32)
            nc.sync.dma_start(out=ib[:], in_=idxb[:])
            ball = cpool.tile([128, _D], bf16)
            nc.gpsimd.indirect_dma_start(
                out=ball[:],
                out_offset=None,
                in_=b_tab[:],
                in_offset=bass.IndirectOffsetOnAxis(ap=ib[:, :1], axis=0),
            )
            b_kt = []
            for k in range(_K):
                btb = cpool.tile([16, _D], bf16, tag=f"bb{k}")
                eng = nc.sync if k % 2 == 0 else nc.scalar
                eng.dma_start(out=btb[:], in_=ball[16 * k : 16 * (k + 1), :])
                b_kt.append(btb)

            # enqueue every remaining z load ahead of all stores in the
            # SWDGE descriptor stream: loads then run back-to-back at
            # full HBM bandwidth while the store backlog drains behind
            zb_pre += [load_zb(k) for k in range(1, _K)]

            # z^T chunks via PE transpose (bf16, 1 cycle/row):
            #   zt[p, dc*512+m] = z[k, m, dc*128+p]
            # Emitted in two halves so the per-k transpose block can be
            # software-pipelined between the previous k's mm1 and mm2:
            # the PE never sees a >2us run of transpose-mode work, which
            # keeps the HAM clock gate warm (transpose-mode doesn't count
            # as PE-busy for HAM).
            zts = {}

            def transposes(k, half):
                zb, zt = zb_pre[k], zts[k]
                for dh in (0, 1) if half == 0 else (2, 3):
                    pt = pst.tile([128, 1024], bf16, tag="zt_ps")
                    for dj in range(2):
                        dc = dh * 2 + dj
                        for mc in range(4):
                            nc.tensor.transpose(
                                out=pt[:, dj * 512 + mc * 128 : dj * 512 + (mc + 1) * 128],
                                in_=zb[:, mc * 1024 + dc * 128 : mc * 1024 + (dc + 1) * 128],
                                identity=ident[:],
                            )
                    dst = zt[:, dh * 1024 : (dh + 1) * 1024]
                    if dh % 2 == 0:
                        nc.vector.tensor_copy(out=dst, in_=pt[:])
                    else:
                        nc.scalar.copy(out=dst, in_=pt[:])

            # ---- main loop over the K selected experts ----
            zt0 = ztpool.tile([128, 4096], bf16, tag="zt")
            zts[0] = zt0
            transposes(0, 0)
            transposes(0, 1)
            for k in range(_K):
                zt = zts[k]
                if k + 1 < _K:
                    ztn = ztpool.tile([128, 4096], bf16, tag="zt")
                    zts[k + 1] = ztn
                    transposes(k + 1, 0)

                # mm1: down^T [16, 512] via one 8-matmul PSUM accumulation
                # group (contiguous in the PE stream - no foreign matmuls
                # can interleave, so hw accumulation is safe here)
                pd = psd.tile([16, _M], f32, tag="down")
                for dc in range(8):
                    nc.tensor.matmul(
                        out=pd[:],
                        lhsT=a_tb[:, (k * 8 + dc) * _R : (k * 8 + dc + 1) * _R],
                        rhs=zt[:, dc * 512 : (dc + 1) * 512],
                        start=(dc == 0),
                        stop=(dc == 7),
                    )
                if k + 1 < _K:
                    transposes(k + 1, 1)

                db = apool.tile([16, _M], bf16, tag="db")
                nc.vector.tensor_copy(out=db[:], in_=pd[:])

                # mm2 + copy out.  f32 staging; the store goes out on the
                # SWDGE queue so it lines up strictly BEHIND all z loads in
                # the descriptor FIFO: loads run at full read bandwidth
                # first, the store backlog drains behind them.
                ov = ovpool.tile([128, 4096], f32, tag="ov")
                for mc in range(4):
                    for dh in range(2):
                        pu = psu.tile([128, 512], f32, tag="up")
                        nc.tensor.matmul(
                            out=pu[:],
                            lhsT=db[:, mc * 128 : (mc + 1) * 128],
                            rhs=b_kt[k][:, dh * 512 : (dh + 1) * 512],
                            start=True,
                            stop=True,
                        )
                        dst = ov[:, mc * 1024 + dh * 512 : mc * 1024 + (dh + 1) * 512]
                        if (mc * 2 + dh) % 2 == 0:
                            nc.vector.tensor_copy(out=dst, in_=pu[:])
                        else:
                            nc.scalar.copy(out=dst, in_=pu[:])
                nc.gpsimd.dma_start(
                    out=out[k].rearrange("(mc p) d -> p mc d", p=128),
                    in_=ov[:].rearrange("p (mc d) -> p mc d", mc=4),
                )

            # trailing no-op SWDGE transfer: keeps the Q7 completion path
            # hot so the final drain sees the last store's semaphore
            # promptly instead of on an idle polling cadence
            tail = cpool.tile([16, 1], i32)
            nc.gpsimd.dma_start(out=tail[:], in_=idxb[0:16, :])
    if split_waits:
        _split_excess_waits(nc)
    return nc


def kernel(z, A_all, B_all, expert_indices, _trace=False):
    from concourse.bass_utils import run_bass_kernel_spmd

    z = np.ascontiguousarray(np.asarray(z, dtype=np.float32))
    A_all = np.ascontiguousarray(np.asarray(A_all, dtype=np.float32))
    B_all = np.ascontiguousarray(np.asarray(B_all, dtype=np.float32))
    idx = np.asarray(expert_indices).astype(np.int64)
    assert z.shape == (_B, _K, _M, _D)

    if "nc" not in _cache:
        _cache["nc"] = _build()
    nc = _cache["nc"]

    a_tab = A_all.reshape(64 * 8, 128 * _R)
    b_tab = B_all.reshape(64 * _R, _D)
    idxa = (idx[:, None] * 8 + np.arange(8)[None, :]).reshape(64, 1).astype(np.int32)
    idxb = (idx[:, None] * 16 + np.arange(16)[None, :]).reshape(128, 1).astype(np.int32)

    in_maps = [
        {"z": z[c], "a_tab": a_tab, "b_tab": b_tab, "idxa": idxa, "idxb": idxb}
        for c in range(_NCORES)
    ]
    res = run_bass_kernel_spmd(nc, in_maps, list(range(_NCORES)), trace=_trace)
    globals()["last_exec_time_ns"] = res.exec_time_ns
    return np.stack([res.results[c]["out"] for c in range(_NCORES)], axis=0)


# revision 55
# speedup vs baseline: 1.1843x; 1.1538x over previous
"""Trainium2 Bass kernel for ExpertMLPLoRA (moe_routing).

Reference computation (per batch b, selected expert k):
    A = A_all[expert_indices]            # [K, D, R]
    Bm = B_all[expert_indices]           # [K, R, D]
    down = einsum('bkmd,kdr->bkmr', z, A)
    up   = einsum('bkmr,krd->bkmd', down, Bm)
    out  = up * (alpha/rank)

Sharding: data-parallel over batch B=8 -> one batch per NeuronCore.
Each core receives its z[b] slice plus the full (replicated) LoRA
tables and gathers the K=8 selected experts on-device via indirect
DMA.  Host only pre-expands the 8 expert indices into row indices
(pure address arithmetic).

Device pipeline per (b, k):
  1. SWDGE cast-DMA z[b,k] [512, 1024] f32 HBM -> bf16 SBUF [128p, (mc,d)]
  2. 32x PE transpose (bf16, via identity matmul) -> z^T chunks in PSUM,
     copied back to SBUF alternating DVE/ACT
  3. mm1: one 8-matmul PSUM accumulation group (start/stop flags) ->
     down^T [16r, 512m] f32 in one PSUM bank; DVE casts to bf16.
     (The accumulation group is contiguous in the PE stream - no
     foreign matmuls interleave, so the hw accumulation fault the old
     partial-add tree worked around cannot trigger.)
  4. mm2: 8 matmuls [16,128m].T @ B_k[16,512d] -> f32 PSUM
  5. PSUM -> SBUF f32 copies (alternating DVE/ACT), SWDGE store per
     128-row strip.

The LoRA scale folds into the bf16 cast of the gathered A table.
"""

import numpy as np

_B, _K, _M, _D, _R = 8, 8, 512, 1024, 16
_SCALE = 1.0 / _R
_NCORES = 8

_cache = {}


def _apply_tile_drain_patch():
    """This walrus build caps sync waits at 1 per instruction (2 for
    EventSemaphore).  Tile's kernel-tail drain piles every final sem wait
    onto one Drain -> NCC_INLA001 'Too many sync wait commands'.  Re-emit
    the extras as standalone per-sem waits before the drain."""
    import concourse.tile as tile_mod
    from concourse.tile import TileContext

    if getattr(TileContext, "_drain_patch_applied", False):
        return
    try:
        from concourse.tile import ScopedClock
    except ImportError:
        from bass_rust import ScopedClock

    def _patched(self, tick_clock, wait_clock):
        nc = self.nc
        probe = nc.sync.drain()
        wait_clock.add_sem_waits(
            probe.ins, ScopedClock({None: tick_clock.global_clock})
        )
        waits = list(probe.ins.sync_info.on_wait)
        if len(waits) > 1:
            assert self.sems is not None
            by_name = {s.name: s for s in self.sems.allocated().values()}
            for w in waits[1:]:
                sem = by_name.get(w.ant_name)
                assert sem is not None, f"semaphore {w.ant_name} not found"
                nc.sync.wait_ge(sem, w.wait_value)
            probe.ins.sync_info.on_wait = waits[:1]
            nc.sync.drain()
        nc.all_engine_barrier()
        assert self.sems is not None
        popped = nc._tile_sem_poison_stack.pop()
        assert popped is self._sem_poison
        nc.clear_and_free_semaphores(list(self.sems.allocated().values()))
        nc.all_engine_barrier()

    TileContext._drain_and_barrier = _patched
    TileContext._drain_patch_applied = True


def _split_excess_waits(nc):
    """This walrus build rejects instructions carrying more than 1-2 sync
    waits ('Too many sync wait commands'), but Tile's sem-assignment packs
    up to ~9 waits onto one instruction.  Hoist the excess onto standalone
    EventSemaphore carriers placed immediately before the instruction on
    the same engine (engines execute in order, so blocking semantics are
    identical)."""
    import bass_rust
    import concourse.mybir as mybir

    n = 0
    for fn in nc.m.functions:
        for bb in fn.blocks:
            new_insts = []
            for inst in bb.instructions:
                si = inst.sync_info
                waits = list(si.on_wait) if si is not None else []
                cap = 2 if isinstance(inst, mybir.InstEventSemaphore) else 1
                if len(waits) > cap:
                    for w in waits[cap:]:
                        n += 1
                        new_insts.append(
                            mybir.InstEventSemaphore(
                                name=f"wsplit-{n}-{inst.name}",
                                engine=inst.engine,
                                ins=[],
                                outs=[],
                                sync_info=bass_rust.SyncInfo(
                                    on_wait=[w], on_update=[]
                                ),
                            )
                        )
                    inst.sync_info = bass_rust.SyncInfo(
                        on_wait=waits[:cap], on_update=list(si.on_update)
                    )
                new_insts.append(inst)
            bb.instructions = new_insts
    return n


def _build(split_waits=True):
    import concourse.bass as bass
    import concourse.mybir as mybir
    from concourse.masks import make_identity
    from concourse.tile import TileContext

    _apply_tile_drain_patch()
    f32 = mybir.dt.float32
    bf16 = mybir.dt.bfloat16
    i32 = mybir.dt.int32

    nc = bass.Bass()
    z = nc.declare_dram_parameter("z", [_K, _M, _D], f32, isOutput=False)
    # A_all [64, 1024, 16] viewed as rows (e, dc) of [128, 16] blocks
    a_tab = nc.declare_dram_parameter("a_tab", [64 * 8, 128 * _R], f32, isOutput=False)
    # B_all [64, 16, 1024] viewed as rows (e, r) of [1024] d-vectors
    b_tab = nc.declare_dram_parameter("b_tab", [64 * _R, _D], f32, isOutput=False)
    idxa = nc.declare_dram_parameter("idxa", [64, 1], i32, isOutput=False)
    idxb = nc.declare_dram_parameter("idxb", [128, 1], i32, isOutput=False)
    out = nc.declare_dram_parameter("out", [_K, _M, _D], f32, isOutput=True)

    with TileContext(nc) as tc:
        with (
            tc.tile_pool(name="const", bufs=1) as cpool,
            tc.tile_pool(name="zbp", bufs=8) as zbpool,
            tc.tile_pool(name="ztp", bufs=2) as ztpool,
            tc.tile_pool(name="ovp", bufs=5) as ovpool,
            tc.tile_pool(name="acc", bufs=2) as apool,
            tc.tile_pool(name="psd", bufs=2, space="PSUM") as psd,
            tc.tile_pool(name="psu", bufs=2, space="PSUM") as psu,
            tc.tile_pool(name="pst", bufs=2, space="PSUM") as pst,
        ):
            def load_zb(k):
                # SWDGE cast-DMA: f32 HBM -> bf16 SBUF in one transfer
                zb = zbpool.tile([128, 4096], bf16, tag="zb")
                nc.gpsimd.dma_start(
                    out=zb[:].rearrange("p (mc d) -> p mc d", mc=4),
                    in_=z[k].rearrange("(mc p) d -> p mc d", p=128),
                )
                return zb

            ident = cpool.tile([128, 128], bf16)
            make_identity(nc, ident[:])

            # first z load starts before any gather work occupies Q7
            zb_pre = [load_zb(0)]

            # ---- one-time expert gather + layout prep ----
            ia = cpool.tile([64, 1], i32)
            nc.sync.dma_start(out=ia[:], in_=idxa[:])

            # gather A rows (k,dc) -> [64, 2048]; row content is [128p, 16r]
            a_rows = cpool.tile([64, 2048], f32)
            nc.gpsimd.indirect_dma_start(
                out=a_rows[:],
                out_offset=None,
                in_=a_tab[:],
                in_offset=bass.IndirectOffsetOnAxis(ap=ia[:, :1], axis=0),
            )
            # identity for PE transposes (f32, A-table prep only)
            identf = cpool.tile([128, 128], f32)
            make_identity(nc, identf[:])
            # redistribute d across partitions with 16 strided PE
            # transposes (one per rank index r): [64(k,dc), 128(d)] -> psum
            # [128(d), (r, k, dc)], then one free-dim-permuted DVE copy with
            # the LoRA scale and bf16 cast folded in.
            a_rows_v = a_rows[:].rearrange("j (p r) -> j r p", r=_R)
            a_tb = cpool.tile([128, 8 * 8 * _R], bf16)
            a_tb_v = a_tb[:].rearrange("p (k dc r) -> p r (k dc)", k=8, r=_R)
            for h in range(2):
                pa = psu.tile([128, 512], f32, tag="up")
                for rr in range(8):
                    nc.tensor.transpose(
                        out=pa[:, rr * 64 : (rr + 1) * 64],
                        in_=a_rows_v[:, h * 8 + rr, :],
                        identity=identf[:64, :64],
                    )
                nc.vector.tensor_scalar_mul(
                    a_tb_v[:, h * 8 : (h + 1) * 8, :],
                    pa[:].rearrange("p (r j) -> p r j", r=8),
                    _SCALE,
                )

            # gather ALL 128 B rows (k,r) in ONE casting indirect DMA
            # (keeps the Q7 descriptor-generation path clear for z loads),
            # then shift each expert's 16 rows to partition base 0 with
            # tiny HWDGE SBUF->SBUF copies (matmul operands must sit at
            # SBUF base partition 0; engines can't shift partitions).
            ib = cpool.tile([128, 1], i32)
            nc.sync.dma_start(out=ib[:], in_=idxb[:])
            ball = cpool.tile([128, _D], bf16)
            nc.gpsimd.indirect_dma_start(
                out=ball[:],
                out_offset=None,
                in_=b_tab[:],
                in_offset=bass.IndirectOffsetOnAxis(ap=ib[:, :1], axis=0),
            )
            b_kt = []
            for k in range(_K):
                btb = cpool.tile([16, _D], bf16, tag=f"bb{k}")
                eng = nc.sync if k % 2 == 0 else nc.scalar
                eng.dma_start(out=btb[:], in_=ball[16 * k : 16 * (k + 1), :])
                b_kt.append(btb)

            # enqueue every remaining z load ahead of all stores in the
            # SWDGE descriptor stream: loads then run back-to-back at
            # full HBM bandwidth while the store backlog drains behind
            zb_pre += [load_zb(k) for k in range(1, _K)]

            # z^T chunks via PE transpose (bf16, 1 cycle/row):
            #   zt[p, dc*512+m] = z[k, m, dc*128+p]
            # Emitted in two halves so the per-k transpose block can be
            # software-pipelined between the previous k's mm1 and mm2:
            # the PE never sees a >2us run of transpose-mode work, which
            # keeps the HAM clock gate warm (transpose-mode doesn't count
            # as PE-busy for HAM).
            zts = {}

            def transposes(k, half):
                zb, zt = zb_pre[k], zts[k]
                for dh in (0, 1) if half == 0 else (2, 3):
                    pt = pst.tile([128, 1024], bf16, tag="zt_ps")
                    for dj in range(2):
                        dc = dh * 2 + dj
                        for mc in range(4):
                            nc.tensor.transpose(
                                out=pt[:, dj * 512 + mc * 128 : dj * 512 + (mc + 1) * 128],
                                in_=zb[:, mc * 1024 + dc * 128 : mc * 1024 + (dc + 1) * 128],
                                identity=ident[:],
                            )
                    dst = zt[:, dh * 1024 : (dh + 1) * 1024]
                    if dh % 2 == 0:
                        nc.vector.tensor_copy(out=dst, in_=pt[:])
                    else:
                        nc.scalar.copy(out=dst, in_=pt[:])

            # ---- main loop over the K selected experts ----
            zt0 = ztpool.tile([128, 4096], bf16, tag="zt")
            zts[0] = zt0
            transposes(0, 0)
            transposes(0, 1)
            for k in range(_K):
                zt = zts[k]
                if k + 1 < _K:
                    ztn = ztpool.tile([128, 4096], bf16, tag="zt")
                    zts[k + 1] = ztn
                    transposes(k + 1, 0)

                # mm1: down^T [16, 512] via one 8-matmul PSUM accumulation
                # group (contiguous in the PE stream - no foreign matmuls
                # can interleave, so hw accumulation is safe here)
                pd = psd.tile([16, _M], f32, tag="down")
                for dc in range(8):
                    nc.tensor.matmul(
                        out=pd[:],
                        lhsT=a_tb[:, (k * 8 + dc) * _R : (k * 8 + dc + 1) * _R],
                        rhs=zt[:, dc * 512 : (dc + 1) * 512],
                        start=(dc == 0),
                        stop=(dc == 7),
                    )
                if k + 1 < _K:
                    transposes(k + 1, 1)

                db = apool.tile([16, _M], bf16, tag="db")
                nc.vector.tensor_copy(out=db[:], in_=pd[:])

                # mm2 + copy out.  f32 staging; the store goes out on the
                # SWDGE queue so it lines up strictly BEHIND all z loads in
                # the descriptor FIFO: loads run at full read bandwidth
                # first, the store backlog drains behind them.
                ov = ovpool.tile([128, 4096], f32, tag="ov")
                for mc in range(4):
                    for dh in range(2):
                        pu = psu.tile([128, 512], f32, tag="up")
                        nc.tensor.matmul(
                            out=pu[:],
                            lhsT=db[:, mc * 128 : (mc + 1) * 128],
                            rhs=b_kt[k][:, dh * 512 : (dh + 1) * 512],
                            start=True,
                            stop=True,
                        )
                        dst = ov[:, mc * 1024 + dh * 512 : mc * 1024 + (dh + 1) * 512]
                        if (mc * 2 + dh) % 2 == 0:
                            nc.vector.tensor_copy(out=dst, in_=pu[:])
                        else:
                            nc.scalar.copy(out=dst, in_=pu[:])
                # stores 0-4: SWDGE, strictly behind the loads in the Q7
                # descriptor FIFO (keeps the load phase read-only).
                # stores 5-7: HWDGE - they run after the loads anyway, and
                # their hardware-posted completion semaphores spare the
                # final drain from SWDGE's lazy completion posting (which
                # showed up as a stochastic ~15us teardown stall).
                if k < 5:
                    seng = nc.gpsimd
                elif k % 2 == 0:
                    seng = nc.sync
                else:
                    seng = nc.scalar
                seng.dma_start(
                    out=out[k].rearrange("(mc p) d -> p mc d", p=128),
                    in_=ov[:].rearrange("p (mc d) -> p mc d", mc=4),
                )

            # trailing no-op SWDGE transfer: keeps the Q7 completion path
            # hot so the last SWDGE store's semaphore posts promptly
            tail = cpool.tile([16, 1], i32)
            nc.gpsimd.dma_start(out=tail[:], in_=idxb[0:16, :])
    if split_waits:
        _split_excess_waits(nc)
    return nc


def kernel(z, A_all, B_all, expert_indices, _trace=False):
    from concourse.bass_utils import run_bass_kernel_spmd

    z = np.ascontiguousarray(np.asarray(z, dtype=np.float32))
    A_all = np.ascontiguousarray(np.asarray(A_all, dtype=np.float32))
    B_all = np.ascontiguousarray(np.asarray(B_all, dtype=np.float32))
    idx = np.asarray(expert_indices).astype(np.int64)
    assert z.shape == (_B, _K, _M, _D)

    if "nc" not in _cache:
        _cache["nc"] = _build()
    nc = _cache["nc"]

    a_tab = A_all.reshape(64 * 8, 128 * _R)
    b_tab = B_all.reshape(64 * _R, _D)
    idxa = (idx[:, None] * 8 + np.arange(8)[None, :]).reshape(64, 1).astype(np.int32)
    idxb = (idx[:, None] * 16 + np.arange(16)[None, :]).reshape(128, 1).astype(np.int32)

    in_maps = [
        {"z": z[c], "a_tab": a_tab, "b_tab": b_tab, "idxa": idxa, "idxb": idxb}
        for c in range(_NCORES)
    ]
    res = run_bass_kernel_spmd(nc, in_maps, list(range(_NCORES)), trace=_trace)
    globals()["last_exec_time_ns"] = res.exec_time_ns
    return np.stack([res.results[c]["out"] for c in range(_NCORES)], axis=0)
